# revision 7
# baseline (speedup 1.0000x reference)
"""TRN2 Bass kernel for channel-attention (dense_transformer, B=8, C=512, T=4096).

Math (per batch element, C=512, T=4096):
    q = Wq x + bq; k = Wk x + bk; v = Wv x + bv          (1x1 convs)
    dots = (q k^T) * SCALE;  attn = softmax(dots, -1);  out = attn v

Gram reformulation (T-contraction happens once):
    dots*SCALE = Wq' G~ Wk'^T with G~ = [x;1][x;1]^T;  out = (attn [Wv|bv]) [x;1]

Quantization: x is split on the host into two fp8-e4m3 terms (global x32
scale): x~ = (a+b)/32.  G = aa^T + ab^T + (ab^T)^T (bb^T dropped), computed
in fp8 DoubleRow at 4x bf16 throughput; G carries a global 2^10 scale that
is folded into Wq' host-side.  Z/dots use the split-bf16 3-term path.
The out matmul runs in fp8 DoubleRow on p1/p2 (exact fp8 split of 512*P)
against the normal-layout fp8 x pair, drained with scale 2^-14.

Host precomputes: all transposes/packing, row-sums xs, the Zr fringe row,
fp8/bf16 weight splits.  Sharding: data-parallel, one batch element/core.
"""
import sys
import numpy as np

for _p in ("/opt/trn_rl_repo", "/root/.axon_site/_ro/trn_rl_repo"):
    if _p not in sys.path:
        sys.path.insert(0, _p)

import ml_dtypes
import concourse.bass as bass
import concourse.tile as tile
import concourse.tile_utils as tile_utils
tile_utils.max_sbuf_usage = 200 * 1024  # cayman: 208KB/partition usable
from concourse import bacc, mybir
from concourse.bass_utils import run_bass_kernel_spmd
from concourse.masks import make_identity

F32 = mybir.dt.float32
BF16 = mybir.dt.bfloat16
FP8 = mybir.dt.float8e4
AF = mybir.ActivationFunctionType
AX = mybir.AxisListType
DR = mybir.MatmulPerfMode.DoubleRow

C = 512
T = 4096
NCH = C // 128   # 4 partition chunks of the channel dim
NTT = T // 128   # 32 t-tiles (transposed layout)
SCALE = np.float32(64 ** -0.5)
GSC = np.float32(2.0 ** -10)   # Gram global scale (a,b carry x32 each)
OSC = np.float32(2.0 ** -14)   # out drain scale (p x512, x-splits x32)

_NC_CACHE = []
_last_in_maps = None

# wblob groups (each [128, 512] bf16): 0-3 wkt_h, 4-7 wkt_l, 8-11 wqt_h',
# 12-15 wqt_l', 16-19 wv, 20 bv (cols 0-3)
W_KH, W_KL, W_QH, W_QL, W_V, W_BV = 0, 4, 8, 12, 16, 20
# sblob cols (f32): bk_bcast*2^10 [0:512], zr_bc [512:1024], xsf [1024:1028],
# bq_col [1028:1032]
S_BK, S_ZR, S_XS, S_BQ = 0, 512, 1024, 1028


def _emit(nc, tc, ctx, d):
    cs = lambda m: slice(128 * m, 128 * (m + 1))

    persist = ctx.enter_context(tc.tile_pool(name="persist", bufs=1))
    work = ctx.enter_context(tc.tile_pool(name="work", bufs=1))
    outp = ctx.enter_context(tc.tile_pool(name="outp", bufs=2))
    psum = ctx.enter_context(tc.tile_pool(name="psum", bufs=8, space="PSUM"))

    # ---- input DMAs (all linear; host did transpose/pack/split) --------
    xta = persist.tile([128, NTT, C], FP8, name="xta", tag="xta")
    xtb = persist.tile([128, NTT, C], FP8, name="xtb", tag="xtb")
    NQ, qt = 4, NTT // 4
    for q in range(NQ):
        qs = slice(q * qt, (q + 1) * qt)
        nc.sync.dma_start(xta[:, qs, :], d["xta"][:, qs, :])
        nc.sync.dma_start(xtb[:, qs, :], d["xtb"][:, qs, :])
    wblob = persist.tile([128, 21, C], BF16, name="wblob", tag="wblob")
    nc.sync.dma_start(wblob[:], d["wblob"][:])
    sblob = persist.tile([128, 1032], F32, name="sblob", tag="sblob")
    nc.sync.dma_start(sblob[:], d["sblob"][:])
    xna = persist.tile([128, NCH, T], FP8, name="xna", tag="xna")
    xnb = persist.tile([128, NCH, T], FP8, name="xnb", tag="xnb")
    nc.sync.dma_start(xna[:], d["xna"][:])
    nc.sync.dma_start(xnb[:], d["xnb"][:])

    # constants
    ident = persist.tile([128, 128], BF16, name="ident", tag="ident")
    make_identity(nc, ident[:])
    one_1 = persist.tile([1, 1], BF16, name="one_1", tag="one_1")
    nc.vector.memset(one_1[:], 1.0)

    # ---- Gram: psG = a a^T, psS = a b^T (fp8 DoubleRow, j-major) -------
    psG = [psum.tile([128, C], F32, name="mm", tag="mm") for _ in range(NCH)]
    psS = [psum.tile([128, C], F32, name="mm", tag="mm") for _ in range(NCH)]
    for j in range(NTT // 2):
        js = slice(2 * j, 2 * j + 2)
        for m in range(NCH):
            nc.tensor.matmul(psG[m][:], xta[:, js, cs(m)], xta[:, js, :],
                             start=(j == 0), stop=False, perf_mode=DR)
            nc.tensor.matmul(psS[m][:], xta[:, js, cs(m)], xtb[:, js, :],
                             start=(j == 0), stop=(j == NTT // 2 - 1),
                             perf_mode=DR)

    # ---- sym: psG += S + S^T; drain to split-bf16 (x2^10 scale) --------
    S_sb = []
    for m in range(NCH):
        s_sb = work.tile([128, C], BF16, name=f"S{m}", tag=f"S{m}")
        nc.scalar.copy(s_sb[:], psS[m][:])
        S_sb.append(s_sb)
    Gh, Gl = [], []
    for m in range(NCH):
        nc.tensor.matmul(psG[m][:], ident[:], S_sb[m][:], start=False, stop=False)
        for j in range(NCH):
            nc.tensor.matmul(psG[m][:, cs(j)], S_sb[j][:, cs(m)], ident[:],
                             start=False, stop=(j == NCH - 1))
        gh = work.tile([128, C], BF16, name=f"Gh{m}", tag=f"Gh{m}")
        gl = work.tile([128, C], BF16, name=f"Gl{m}", tag=f"Gl{m}")
        nc.scalar.copy(gh[:], psG[m][:])
        nc.vector.tensor_sub(gl[:], psG[m][:], gh[:])
        Gh.append(gh)
        Gl.append(gl)

    # ---- Z = G~ Wk'^T (split-bf16 3-term) + rank-1 bias fringe ---------
    Zh, Zl = [], []
    for m in range(NCH):
        ps = psum.tile([128, C], F32, name="mm", tag="mm")
        first = True
        cnt = 0
        for k in range(NCH):
            for lh, wg in ((Gh[k], W_KH), (Gh[k], W_KL), (Gl[k], W_KH)):
                cnt += 1
                nc.tensor.matmul(ps[:], lh[:, cs(m)], wblob[:, wg + k, :],
                                 start=first, stop=(cnt == 3 * NCH))
                first = False
        # fringe xs[c]*bk[d]*2^10 folded in on DVE, in place on the psum
        nc.vector.scalar_tensor_tensor(ps[:], sblob[:, S_BK:S_BK + C],
                                       sblob[:, S_XS + m:S_XS + m + 1],
                                       ps[:], op0=mybir.AluOpType.mult,
                                       op1=mybir.AluOpType.add)
        zh = work.tile([128, C], BF16, name=f"Zh{m}", tag=f"Zh{m}")
        zl = work.tile([128, C], BF16, name=f"Zl{m}", tag=f"Zl{m}")
        nc.scalar.copy(zh[:], ps[:])
        nc.vector.tensor_sub(zl[:], ps[:], zh[:])
        Zh.append(zh)
        Zl.append(zl)

    # ---- dots = Wq'' Z~ (2^-10 folded into Wq'') ; fused softmax -------
    attn_un, diag = [], []
    for m in range(NCH):
        ps = psum.tile([128, C], F32, name="mm", tag="mm")
        first = True
        cnt = 0
        for k in range(NCH):
            for wg, rh in ((W_QH, Zh[k]), (W_QH, Zl[k]), (W_QL, Zh[k])):
                cnt += 1
                nc.tensor.matmul(ps[:], wblob[:, wg + k, cs(m)], rh[:],
                                 start=first, stop=(cnt == 3 * NCH))
                first = False
        # fringe bq[c]*Zr[d] on DVE (zr_bc is f32, exact)
        nc.vector.scalar_tensor_tensor(ps[:], sblob[:, S_ZR:S_ZR + C],
                                       sblob[:, S_BQ + m:S_BQ + m + 1],
                                       ps[:], op0=mybir.AluOpType.mult,
                                       op1=mybir.AluOpType.add)
        nmx = work.tile([128, 1], F32, name=f"nmx{m}", tag=f"nmx{m}")
        nc.vector.reduce_max(nmx[:], ps[:], axis=AX.X, negate=True)
        au = work.tile([128, C], BF16, name=f"au{m}", tag=f"au{m}")
        sm = work.tile([128, 1], F32, name=f"sm{m}", tag=f"sm{m}")
        nc.vector.memset(sm[:], 0.0)
        nc.scalar.activation(au[:], ps[:], AF.Exp, bias=nmx[:], scale=1.0,
                             accum_out=sm[:])
        ri = work.tile([128, 1], F32, name=f"ri{m}", tag=f"ri{m}")
        nc.vector.reciprocal(ri[:], sm[:])
        dg = work.tile([128, 128], BF16, name=f"dg{m}", tag=f"dg{m}")
        nc.vector.tensor_scalar_mul(dg[:], ident[:], ri[:])
        attn_un.append(au)
        diag.append(dg)

    # ---- attn^T (normalized) via matmul with diag(1/sum) rhs -----------
    attnT = []
    for j in range(NCH):
        ps = psum.tile([128, C], F32, name="mm", tag="mm")
        for m in range(NCH):
            nc.tensor.matmul(ps[:, cs(m)], attn_un[m][:, cs(j)], diag[m][:],
                             start=True, stop=True)
        at = work.tile([128, C], BF16, name=f"at{j}", tag=f"at{j}")
        nc.scalar.copy(at[:], ps[:])
        attnT.append(at)

    # ---- P~^T = [Wv|bv]^T attn^T ; exact fp8 split of 512*P ------------
    p1 = work.tile([128, NCH, C], FP8, name="p1", tag="p1")
    p2 = work.tile([128, NCH, C], FP8, name="p2", tag="p2")
    for jm in range(NCH):
        ps = psum.tile([128, C], F32, name="mm", tag="mm")
        for k in range(NCH):
            nc.tensor.matmul(ps[:], wblob[:, W_V + k, cs(jm)], attnT[k][:],
                             start=(k == 0), stop=(k == NCH - 1))
        pt = work.tile([128, C], BF16, name=f"pt{jm}", tag=f"pt{jm}")
        nc.scalar.activation(pt[:], ps[:], AF.Copy, scale=512.0)
        nc.vector.tensor_copy(p1[:, jm, :], pt[:])
        nc.vector.tensor_sub(p2[:, jm, :], pt[:], p1[:, jm, :])
    # r = attn bv  (as a [1, C] row), then transposed to per-chunk [128, 1]
    ps = psum.tile([1, C], F32, name="mm", tag="mm")
    for k in range(NCH):
        nc.tensor.matmul(ps[:], wblob[:, W_BV, k:k + 1], attnT[k][:],
                         start=(k == 0), stop=(k == NCH - 1))
    r_b = work.tile([1, C], BF16, name="rb", tag="rb")
    nc.scalar.copy(r_b[:], ps[:])
    rT = []
    ps_rt = psum.tile([128, NCH], F32, name="mm", tag="mm")
    for m in range(NCH):
        nc.tensor.matmul(ps_rt[:, m:m + 1], r_b[:, cs(m)], one_1[:],
                         start=True, stop=True)
    for m in range(NCH):
        rt = work.tile([128, 1], F32, name=f"rT{m}", tag=f"rT{m}")
        nc.vector.tensor_copy(rt[:], ps_rt[:, m:m + 1])
        rT.append(rt)

    # ---- out = (p1(a+b) + p2 a) * 2^-14 + r  (fp8 DoubleRow) -----------
    for m in range(NCH):
        for h in range(2):
            ob = outp.tile([128, 2048], F32, name="ob", tag="ob")
            for t4 in range(4):
                sl = slice(2048 * h + 512 * t4, 2048 * h + 512 * (t4 + 1))
                ps = psum.tile([128, 512], F32, name="mm", tag="mm")
                cnt = 0
                for kp in range(NCH // 2):
                    ks = slice(2 * kp, 2 * kp + 2)
                    for lh, rh in ((p1, xna), (p1, xnb), (p2, xna)):
                        cnt += 1
                        nc.tensor.matmul(
                            ps[:], lh[:, ks, cs(m)], rh[:, ks, sl],
                            start=(cnt == 1), stop=(cnt == 6), perf_mode=DR)
                nc.scalar.activation(ob[:, 512 * t4:512 * (t4 + 1)], ps[:],
                                     AF.Identity, bias=rT[m][:], scale=float(OSC))
            nc.sync.dma_start(d["out"][cs(m), 2048 * h:2048 * (h + 1)], ob[:])


def _declare(nc):
    d = {}
    d["xta"] = nc.declare_dram_parameter("xta", [128, NTT, C], FP8, isOutput=False)
    d["xtb"] = nc.declare_dram_parameter("xtb", [128, NTT, C], FP8, isOutput=False)
    d["xna"] = nc.declare_dram_parameter("xna", [128, NCH, T], FP8, isOutput=False)
    d["xnb"] = nc.declare_dram_parameter("xnb", [128, NCH, T], FP8, isOutput=False)
    d["wblob"] = nc.declare_dram_parameter("wblob", [128, 21, C], BF16, isOutput=False)
    d["sblob"] = nc.declare_dram_parameter("sblob", [128, 1032], F32, isOutput=False)
    d["out"] = nc.declare_dram_parameter("out", [C, T], F32, isOutput=True)
    return d


def _build_nc():
    from contextlib import ExitStack
    nc = bacc.Bacc()
    d = _declare(nc)
    with tile.TileContext(nc) as tc:
        with ExitStack() as ctx:
            _emit(nc, tc, ctx, d)
    nc.finalize()
    return nc


def _f8(a):
    return a.astype(ml_dtypes.float8_e4m3)


def _bf(a):
    return a.astype(ml_dtypes.bfloat16)


def _split_bf16(a):
    h = _bf(a)
    l = _bf(a.astype(np.float32) - h.astype(np.float32))
    return h, l


def kernel(x, Wq, bq, Wk, bk, Wv, bv):
    x = np.ascontiguousarray(np.asarray(x, dtype=np.float32))
    B = x.shape[0]
    assert x.shape == (B, C, T)

    wkt = np.ascontiguousarray(Wk.T.astype(np.float32))
    wqt = np.ascontiguousarray(Wq.T.astype(np.float32) * SCALE * GSC)
    wkt_h, wkt_l = _split_bf16(wkt)
    wqt_h, wqt_l = _split_bf16(wqt)
    wv_b = _bf(Wv.astype(np.float32))

    def chunks(a):  # [C, C] -> [4][128, C] stacked into [128, 4, C]
        return np.ascontiguousarray(
            a.reshape(NCH, 128, -1).transpose(1, 0, 2))

    wblob = np.zeros((128, 21, C), ml_dtypes.bfloat16)
    wblob[:, W_KH:W_KH + 4, :] = chunks(wkt_h)
    wblob[:, W_KL:W_KL + 4, :] = chunks(wkt_l)
    wblob[:, W_QH:W_QH + 4, :] = chunks(wqt_h)
    wblob[:, W_QL:W_QL + 4, :] = chunks(wqt_l)
    wblob[:, W_V:W_V + 4, :] = chunks(wv_b)
    wblob[:, W_BV, :NCH] = _bf(bv.astype(np.float32)).reshape(NCH, 128).T

    in_maps = []
    for b in range(B):
        xb = x[b]
        xs32 = xb * np.float32(32.0)
        a8 = _f8(xs32)
        b8 = _f8(xs32 - a8.astype(np.float32))
        # transposed pack: [C, T] -> x^T [T, C] -> [128, NTT, C]
        def tpack(v):
            return np.ascontiguousarray(
                v.T.reshape(NTT, 128, C).transpose(1, 0, 2))
        # normal pack: [C, T] -> [128, NCH, T]
        def npack(v):
            return np.ascontiguousarray(
                v.reshape(NCH, 128, T).transpose(1, 0, 2))
        xsum = xb.sum(axis=1, dtype=np.float64).astype(np.float32)
        zr = (xsum @ wkt + np.float32(T) * bk.astype(np.float32)).astype(
            np.float32)
        sblob = np.zeros((128, 1032), np.float32)
        sblob[:, S_BK:S_BK + C] = np.broadcast_to(
            bk.astype(np.float32)[None, :] * np.float32(1.0 / GSC), (128, C))
        sblob[:, S_ZR:S_ZR + C] = np.broadcast_to(zr[None, :], (128, C))
        sblob[:, S_XS:S_XS + NCH] = xsum.reshape(NCH, 128).T
        sblob[:, S_BQ:S_BQ + NCH] = (bq.astype(np.float32) * SCALE).reshape(
            NCH, 128).T
        in_maps.append(dict(
            xta=tpack(a8), xtb=tpack(b8), xna=npack(a8), xnb=npack(b8),
            wblob=wblob, sblob=sblob))

    if not _NC_CACHE:
        _NC_CACHE.append(_build_nc())
    nc = _NC_CACHE[0]

    global _last_in_maps
    _last_in_maps = in_maps

    res = run_bass_kernel_spmd(nc, in_maps, list(range(B)))
    return np.stack([res.results[b]["out"] for b in range(B)], axis=0)
